# revision 19
# baseline (speedup 1.0000x reference)
"""AdEx neuron RHS on 8 Trainium2 NeuronCores (Bass/Tile, SPMD).

dVdt = (-(V - V_rest) + delta_T*exp((V - V_T)/delta_T) - R*w + R*I(t)) / tau
dwdt = (a*(V - V_rest) - w) / tau_w

All [1]-shaped params plus the I_ext(t) table lookup are folded on the host
into 8 scalar constants, so the device kernel is pure elementwise:

    E  = exp(s_exp*V + b_exp)          # == (delta_T/tau)*exp((V-V_T)/delta_T)
    dV = alpha*V + (beta*w + gamma) + E
    dw = a2*V + (b2*w + c2w)

Sharding: V/w (and both outputs) split evenly across 8 cores on axis 0;
the constants are replicated.
"""

import math

import numpy as np

N = 33554432
NCORES = 8
NSHARD = N // NCORES  # 4194304
P = 128
FD = 2048  # default free-dim elements per tile
I_BIN = 0.001

_BUILT = {}


def _build(consts, repeat=1, mode="full", fd=None, bufs=3):
    """consts: tuple of 8 f32 floats (s_exp, b_exp, b2, c2w, beta, gamma, a2, alpha).

    repeat>1 wraps the whole shard pass in a dynamic For_i loop (for slope
    benchmarking: per-pass time = d(wall)/d(repeat), immune to dispatch
    overhead). mode="memcpy" skips compute (DMA roundtrip probe)."""
    fd = FD if fd is None else fd
    key = (consts, repeat, mode, fd, bufs)
    if key in _BUILT:
        return _BUILT[key]
    ntiles = NSHARD // (P * fd)

    import concourse.bacc as bacc
    import concourse.mybir as mybir
    from concourse.tile import TileContext

    f32 = mybir.dt.float32
    AF = mybir.ActivationFunctionType
    OP = mybir.AluOpType
    s_exp, b_exp, b_w2, c_w2, s_q, b_q, a2, alpha = consts

    nc = bacc.Bacc(None)
    V = nc.declare_dram_parameter("V", [NSHARD], f32, isOutput=False)
    w = nc.declare_dram_parameter("w", [NSHARD], f32, isOutput=False)
    dV = nc.declare_dram_parameter("dVdt", [NSHARD], f32, isOutput=True)
    dw = nc.declare_dram_parameter("dwdt", [NSHARD], f32, isOutput=True)

    V3 = V[:].rearrange("(n p m) -> n p m", p=P, m=fd)
    w3 = w[:].rearrange("(n p m) -> n p m", p=P, m=fd)
    dV3 = dV[:].rearrange("(n p m) -> n p m", p=P, m=fd)
    dw3 = dw[:].rearrange("(n p m) -> n p m", p=P, m=fd)

    # Exp's bias must be a per-partition SBUF AP (walrus requirement for
    # non-Copy activations); memset one before the Tile region, like Bass's
    # own const-AP registration does.
    bexp_t = nc.alloc_sbuf_tensor("const-bexp", [P, 1], f32)
    nc.gpsimd.memset(bexp_t.ap(), b_exp)
    nc.all_engine_barrier()
    b_exp_ap = bexp_t.ap()

    with TileContext(nc) as tc:
        with tc.tile_pool(name="pool", bufs=bufs) as pool:

            def body():
                for i in range(ntiles):
                    vt = pool.tile([P, fd], f32)
                    nc.sync.dma_start(out=vt[:, :], in_=V3[i, :, :])
                    wt = pool.tile([P, fd], f32)
                    nc.sync.dma_start(out=wt[:, :], in_=w3[i, :, :])

                    if mode == "memcpy":
                        nc.sync.dma_start(out=dV3[i, :, :], in_=vt[:, :])
                        nc.sync.dma_start(out=dw3[i, :, :], in_=wt[:, :])
                        continue

                    # E = (delta_T/tau) * exp((V-V_T)/delta_T)   [ScalarE]
                    et = pool.tile([P, fd], f32)
                    nc.scalar.activation(
                        et[:, :], vt[:, :], AF.Exp, bias=b_exp_ap, scale=s_exp
                    )
                    # at = alpha*V + gamma                        [DVE TS 2x]
                    at = pool.tile([P, fd], f32)
                    nc.vector.tensor_scalar(
                        at[:, :], vt[:, :], alpha, b_q, OP.mult, OP.add
                    )
                    # w2t = b2*w                                  [ScalarE]
                    w2t = pool.tile([P, fd], f32)
                    nc.scalar.activation(
                        w2t[:, :], wt[:, :], AF.Copy, bias=0.0, scale=b_w2
                    )
                    # vt := a2*V + c2w  (in-place; V fully consumed) [DVE TS 2x]
                    nc.vector.tensor_scalar(
                        vt[:, :], vt[:, :], a2, c_w2, OP.mult, OP.add
                    )
                    # wt := beta*w  (in-place; w fully consumed)  [ScalarE]
                    nc.scalar.activation(
                        wt[:, :], wt[:, :], AF.Copy, bias=0.0, scale=s_q
                    )
                    # at += beta*w ; at += E  → dVdt              [DVE TT 1x]
                    nc.vector.tensor_add(out=at[:, :], in0=at[:, :], in1=wt[:, :])
                    nc.vector.tensor_add(out=at[:, :], in0=at[:, :], in1=et[:, :])
                    # vt += b2*w → dwdt                           [GpSimd TT]
                    nc.gpsimd.tensor_add(out=vt[:, :], in0=vt[:, :], in1=w2t[:, :])

                    nc.sync.dma_start(out=dV3[i, :, :], in_=at[:, :])
                    nc.sync.dma_start(out=dw3[i, :, :], in_=vt[:, :])

            if repeat == 1:
                body()
            else:
                with tc.For_i(0, repeat, 1):
                    body()

    if not nc.is_finalized():
        nc.finalize()  # Bacc.finalize runs compile() (reg alloc, wait splitting)
    _BUILT[key] = nc
    return nc


def _fold_constants(inputs):
    t = np.asarray(inputs["t"], dtype=np.float32)
    I_ext = np.asarray(inputs["I_ext"], dtype=np.float32)
    scal = lambda k: float(np.asarray(inputs[k]).reshape(-1)[0])
    V_rest, V_T, delta_T = scal("V_rest"), scal("V_T"), scal("delta_T")
    R, tau, tau_w, a = scal("R"), scal("tau"), scal("tau_w"), scal("a")

    # idx exactly as the reference: floor(t[0]/I_BIN) in f32
    idx = int(np.floor(np.divide(t[0], np.float32(I_BIN), dtype=np.float32)))
    I_t = float(I_ext[idx])

    s_exp = 1.0 / delta_T
    b_exp = -V_T / delta_T + math.log(delta_T / tau)
    alpha = -1.0 / tau
    beta = -R / tau
    gamma = (V_rest + R * I_t) / tau
    a2 = a / tau_w
    b2 = -1.0 / tau_w
    c2w = -a * V_rest / tau_w

    row = np.array([s_exp, b_exp, b2, c2w, beta, gamma, a2, alpha], dtype=np.float32)
    return tuple(float(x) for x in row)


def run(inputs, trace=False, **kwargs):
    """Compile+run on 8 cores; returns ((dVdt, dwdt), BassKernelResults)."""
    from concourse.bass_utils import run_bass_kernel_spmd

    V = np.ascontiguousarray(np.asarray(inputs["V"], dtype=np.float32))
    w = np.ascontiguousarray(np.asarray(inputs["w"], dtype=np.float32))
    consts = _fold_constants(inputs)

    nc = _build(consts)
    in_maps = [
        {
            "V": V[c * NSHARD : (c + 1) * NSHARD],
            "w": w[c * NSHARD : (c + 1) * NSHARD],
        }
        for c in range(NCORES)
    ]
    res = run_bass_kernel_spmd(nc, in_maps, list(range(NCORES)), trace=trace, **kwargs)
    dVdt = np.concatenate([res.results[c]["dVdt"] for c in range(NCORES)])
    dwdt = np.concatenate([res.results[c]["dwdt"] for c in range(NCORES)])
    return (dVdt, dwdt), res


def kernel(**inputs):
    out, _ = run(inputs, trace=False)
    return out


def make_exec_fn(consts, repeat=1, mode="full", fd=None, bufs=3):
    """Build a reusable jitted executor over pre-sharded device arrays.

    Returns (fn, mesh, arg_names): fn(*dev_arrays) -> outputs; arg order is
    V_full, w_full, dV_zeros, dw_zeros (each a full [N] array sharded on
    axis 0 across the 8-core mesh). For slope benchmarking only.
    """
    import jax
    from jax.experimental.shard_map import shard_map
    from jax.sharding import Mesh, PartitionSpec

    from concourse import bass2jax, mybir

    bass2jax.install_neuronx_cc_hook()
    nc = _build(consts, repeat, mode, fd=fd, bufs=bufs)

    partition_name = nc.partition_id_tensor.name if nc.partition_id_tensor else None
    in_names, out_names, out_avals = [], [], []
    for alloc in nc.m.functions[0].allocations:
        if not isinstance(alloc, mybir.MemoryLocationSet):
            continue
        name = alloc.memorylocations[0].name
        if alloc.kind == "ExternalInput":
            if name != partition_name:
                in_names.append(name)
        elif alloc.kind == "ExternalOutput":
            out_names.append(name)
            out_avals.append(
                jax.core.ShapedArray(
                    tuple(alloc.tensor_shape), mybir.dt.np(alloc.dtype)
                )
            )
    all_in = list(in_names) + list(out_names)
    if partition_name is not None:
        all_in.append(partition_name)

    def _body(*args):
        operands = list(args)
        if partition_name is not None:
            operands.append(bass2jax.partition_id_tensor())
        outs = bass2jax._bass_exec_p.bind(
            *operands,
            out_avals=tuple(out_avals),
            in_names=tuple(all_in),
            out_names=tuple(out_names),
            lowering_input_output_aliases=(),
            sim_require_finite=True,
            sim_require_nnan=True,
            nc=nc,
        )
        return tuple(outs)

    devices = jax.devices()[: NCORES]
    mesh = Mesh(np.asarray(devices), ("core",))
    nargs = len(in_names) + len(out_names)
    fn = jax.jit(
        shard_map(
            _body,
            mesh=mesh,
            in_specs=(PartitionSpec("core"),) * nargs,
            out_specs=(PartitionSpec("core"),) * len(out_names),
            check_rep=False,
        ),
        keep_unused=True,
    )
    return fn, mesh, in_names + out_names


# revision 20
# speedup vs baseline: 1.0823x; 1.0823x over previous
"""AdEx neuron RHS on 8 Trainium2 NeuronCores (Bass/Tile, SPMD).

dVdt = (-(V - V_rest) + delta_T*exp((V - V_T)/delta_T) - R*w + R*I(t)) / tau
dwdt = (a*(V - V_rest) - w) / tau_w

All [1]-shaped params plus the I_ext(t) table lookup are folded on the host
into 8 scalar constants, so the device kernel is pure elementwise:

    E  = exp(s_exp*V + b_exp)          # == (delta_T/tau)*exp((V-V_T)/delta_T)
    dV = alpha*V + (beta*w + gamma) + E
    dw = a2*V + (b2*w + c2w)

Sharding: V/w (and both outputs) split evenly across 8 cores on axis 0;
the constants are replicated.
"""

import math

import numpy as np

N = 33554432
NCORES = 8
NSHARD = N // NCORES  # 4194304
P = 128
FD = 2048  # default free-dim elements per tile
I_BIN = 0.001

_BUILT = {}


def _build(consts, repeat=1, mode="full", fd=None, bufs=3):
    """consts: tuple of 8 f32 floats (s_exp, b_exp, b2, c2w, beta, gamma, a2, alpha).

    repeat>1 wraps the whole shard pass in a dynamic For_i loop (for slope
    benchmarking: per-pass time = d(wall)/d(repeat), immune to dispatch
    overhead). mode="memcpy" skips compute (DMA roundtrip probe)."""
    fd = FD if fd is None else fd
    key = (consts, repeat, mode, fd, bufs)
    if key in _BUILT:
        return _BUILT[key]
    ntiles = NSHARD // (P * fd)

    import concourse.bacc as bacc
    import concourse.mybir as mybir
    from concourse.tile import TileContext

    f32 = mybir.dt.float32
    AF = mybir.ActivationFunctionType
    OP = mybir.AluOpType
    s_exp, b_exp, b_w2, c_w2, s_q, b_q, a2, alpha = consts

    nc = bacc.Bacc(None)
    V = nc.declare_dram_parameter("V", [NSHARD], f32, isOutput=False)
    w = nc.declare_dram_parameter("w", [NSHARD], f32, isOutput=False)
    dV = nc.declare_dram_parameter("dVdt", [NSHARD], f32, isOutput=True)
    dw = nc.declare_dram_parameter("dwdt", [NSHARD], f32, isOutput=True)

    V3 = V[:].rearrange("(n p m) -> n p m", p=P, m=fd)
    w3 = w[:].rearrange("(n p m) -> n p m", p=P, m=fd)
    dV3 = dV[:].rearrange("(n p m) -> n p m", p=P, m=fd)
    dw3 = dw[:].rearrange("(n p m) -> n p m", p=P, m=fd)

    # Exp's bias must be a per-partition SBUF AP (walrus requirement for
    # non-Copy activations); memset one before the Tile region, like Bass's
    # own const-AP registration does.
    bexp_t = nc.alloc_sbuf_tensor("const-bexp", [P, 1], f32)
    nc.gpsimd.memset(bexp_t.ap(), b_exp)
    nc.all_engine_barrier()
    b_exp_ap = bexp_t.ap()

    with TileContext(nc) as tc:
        with tc.tile_pool(name="pool", bufs=bufs) as pool:

            def body():
                for i in range(ntiles):
                    vt = pool.tile([P, fd], f32)
                    nc.sync.dma_start(out=vt[:, :], in_=V3[i, :, :])
                    wt = pool.tile([P, fd], f32)
                    nc.sync.dma_start(out=wt[:, :], in_=w3[i, :, :])

                    if mode == "memcpy":
                        nc.sync.dma_start(out=dV3[i, :, :], in_=vt[:, :])
                        nc.sync.dma_start(out=dw3[i, :, :], in_=wt[:, :])
                        continue

                    # E = (delta_T/tau) * exp((V-V_T)/delta_T)   [ScalarE]
                    et = pool.tile([P, fd], f32)
                    nc.scalar.activation(
                        et[:, :], vt[:, :], AF.Exp, bias=b_exp_ap, scale=s_exp
                    )
                    # at = alpha*V + gamma                        [DVE TS 2x]
                    at = pool.tile([P, fd], f32)
                    nc.vector.tensor_scalar(
                        at[:, :], vt[:, :], alpha, b_q, OP.mult, OP.add
                    )
                    # w2t = b2*w                                  [ScalarE]
                    w2t = pool.tile([P, fd], f32)
                    nc.scalar.activation(
                        w2t[:, :], wt[:, :], AF.Copy, bias=0.0, scale=b_w2
                    )
                    # vt := a2*V + c2w  (in-place; V fully consumed) [DVE TS 2x]
                    nc.vector.tensor_scalar(
                        vt[:, :], vt[:, :], a2, c_w2, OP.mult, OP.add
                    )
                    # wt := beta*w  (in-place; w fully consumed)  [ScalarE]
                    nc.scalar.activation(
                        wt[:, :], wt[:, :], AF.Copy, bias=0.0, scale=s_q
                    )
                    # at += beta*w ; at += E  → dVdt              [DVE TT 1x]
                    nc.vector.tensor_add(out=at[:, :], in0=at[:, :], in1=wt[:, :])
                    nc.vector.tensor_add(out=at[:, :], in0=at[:, :], in1=et[:, :])
                    # vt += b2*w → dwdt                           [DVE TT 1x]
                    nc.vector.tensor_add(out=vt[:, :], in0=vt[:, :], in1=w2t[:, :])

                    nc.sync.dma_start(out=dV3[i, :, :], in_=at[:, :])
                    nc.sync.dma_start(out=dw3[i, :, :], in_=vt[:, :])

            if repeat == 1:
                body()
            else:
                with tc.For_i(0, repeat, 1):
                    body()

    if not nc.is_finalized():
        nc.finalize()  # Bacc.finalize runs compile() (reg alloc, wait splitting)
    _BUILT[key] = nc
    return nc


def _fold_constants(inputs):
    t = np.asarray(inputs["t"], dtype=np.float32)
    I_ext = np.asarray(inputs["I_ext"], dtype=np.float32)
    scal = lambda k: float(np.asarray(inputs[k]).reshape(-1)[0])
    V_rest, V_T, delta_T = scal("V_rest"), scal("V_T"), scal("delta_T")
    R, tau, tau_w, a = scal("R"), scal("tau"), scal("tau_w"), scal("a")

    # idx exactly as the reference: floor(t[0]/I_BIN) in f32
    idx = int(np.floor(np.divide(t[0], np.float32(I_BIN), dtype=np.float32)))
    I_t = float(I_ext[idx])

    s_exp = 1.0 / delta_T
    b_exp = -V_T / delta_T + math.log(delta_T / tau)
    alpha = -1.0 / tau
    beta = -R / tau
    gamma = (V_rest + R * I_t) / tau
    a2 = a / tau_w
    b2 = -1.0 / tau_w
    c2w = -a * V_rest / tau_w

    row = np.array([s_exp, b_exp, b2, c2w, beta, gamma, a2, alpha], dtype=np.float32)
    return tuple(float(x) for x in row)


def run(inputs, trace=False, **kwargs):
    """Compile+run on 8 cores; returns ((dVdt, dwdt), BassKernelResults)."""
    from concourse.bass_utils import run_bass_kernel_spmd

    V = np.ascontiguousarray(np.asarray(inputs["V"], dtype=np.float32))
    w = np.ascontiguousarray(np.asarray(inputs["w"], dtype=np.float32))
    consts = _fold_constants(inputs)

    nc = _build(consts)
    in_maps = [
        {
            "V": V[c * NSHARD : (c + 1) * NSHARD],
            "w": w[c * NSHARD : (c + 1) * NSHARD],
        }
        for c in range(NCORES)
    ]
    res = run_bass_kernel_spmd(nc, in_maps, list(range(NCORES)), trace=trace, **kwargs)
    dVdt = np.concatenate([res.results[c]["dVdt"] for c in range(NCORES)])
    dwdt = np.concatenate([res.results[c]["dwdt"] for c in range(NCORES)])
    return (dVdt, dwdt), res


def kernel(**inputs):
    out, _ = run(inputs, trace=False)
    return out


def make_exec_fn(consts, repeat=1, mode="full", fd=None, bufs=3):
    """Build a reusable jitted executor over pre-sharded device arrays.

    Returns (fn, mesh, arg_names): fn(*dev_arrays) -> outputs; arg order is
    V_full, w_full, dV_zeros, dw_zeros (each a full [N] array sharded on
    axis 0 across the 8-core mesh). For slope benchmarking only.
    """
    import jax
    from jax.experimental.shard_map import shard_map
    from jax.sharding import Mesh, PartitionSpec

    from concourse import bass2jax, mybir

    bass2jax.install_neuronx_cc_hook()
    nc = _build(consts, repeat, mode, fd=fd, bufs=bufs)

    partition_name = nc.partition_id_tensor.name if nc.partition_id_tensor else None
    in_names, out_names, out_avals = [], [], []
    for alloc in nc.m.functions[0].allocations:
        if not isinstance(alloc, mybir.MemoryLocationSet):
            continue
        name = alloc.memorylocations[0].name
        if alloc.kind == "ExternalInput":
            if name != partition_name:
                in_names.append(name)
        elif alloc.kind == "ExternalOutput":
            out_names.append(name)
            out_avals.append(
                jax.core.ShapedArray(
                    tuple(alloc.tensor_shape), mybir.dt.np(alloc.dtype)
                )
            )
    all_in = list(in_names) + list(out_names)
    if partition_name is not None:
        all_in.append(partition_name)

    def _body(*args):
        operands = list(args)
        if partition_name is not None:
            operands.append(bass2jax.partition_id_tensor())
        outs = bass2jax._bass_exec_p.bind(
            *operands,
            out_avals=tuple(out_avals),
            in_names=tuple(all_in),
            out_names=tuple(out_names),
            lowering_input_output_aliases=(),
            sim_require_finite=True,
            sim_require_nnan=True,
            nc=nc,
        )
        return tuple(outs)

    devices = jax.devices()[: NCORES]
    mesh = Mesh(np.asarray(devices), ("core",))
    nargs = len(in_names) + len(out_names)
    fn = jax.jit(
        shard_map(
            _body,
            mesh=mesh,
            in_specs=(PartitionSpec("core"),) * nargs,
            out_specs=(PartitionSpec("core"),) * len(out_names),
            check_rep=False,
        ),
        keep_unused=True,
    )
    return fn, mesh, in_names + out_names


# revision 21
# speedup vs baseline: 1.1061x; 1.0219x over previous
"""AdEx neuron RHS on 8 Trainium2 NeuronCores (Bass/Tile, SPMD).

dVdt = (-(V - V_rest) + delta_T*exp((V - V_T)/delta_T) - R*w + R*I(t)) / tau
dwdt = (a*(V - V_rest) - w) / tau_w

All [1]-shaped params plus the I_ext(t) table lookup are folded on the host
into 8 scalar constants, so the device kernel is pure elementwise:

    E  = exp(s_exp*V + b_exp)          # == (delta_T/tau)*exp((V-V_T)/delta_T)
    dV = alpha*V + (beta*w + gamma) + E
    dw = a2*V + (b2*w + c2w)

Sharding: V/w (and both outputs) split evenly across 8 cores on axis 0;
the constants are replicated.
"""

import math

import numpy as np

N = 33554432
NCORES = 8
NSHARD = N // NCORES  # 4194304
P = 128
FD = 2048  # default free-dim elements per tile
I_BIN = 0.001

_BUILT = {}


def _build(consts, repeat=1, mode="full", fd=None, bufs=3):
    """consts: tuple of 8 f32 floats (s_exp, b_exp, b2, c2w, beta, gamma, a2, alpha).

    repeat>1 wraps the whole shard pass in a dynamic For_i loop (for slope
    benchmarking: per-pass time = d(wall)/d(repeat), immune to dispatch
    overhead). mode="memcpy" skips compute (DMA roundtrip probe)."""
    fd = FD if fd is None else fd
    key = (consts, repeat, mode, fd, bufs)
    if key in _BUILT:
        return _BUILT[key]
    ntiles = NSHARD // (P * fd)

    import concourse.bacc as bacc
    import concourse.mybir as mybir
    from concourse.tile import TileContext

    f32 = mybir.dt.float32
    AF = mybir.ActivationFunctionType
    OP = mybir.AluOpType
    s_exp, b_exp, b_w2, c_w2, s_q, b_q, a2, alpha = consts

    nc = bacc.Bacc(None)
    V = nc.declare_dram_parameter("V", [NSHARD], f32, isOutput=False)
    w = nc.declare_dram_parameter("w", [NSHARD], f32, isOutput=False)
    dV = nc.declare_dram_parameter("dVdt", [NSHARD], f32, isOutput=True)
    dw = nc.declare_dram_parameter("dwdt", [NSHARD], f32, isOutput=True)

    V3 = V[:].rearrange("(n p m) -> n p m", p=P, m=fd)
    w3 = w[:].rearrange("(n p m) -> n p m", p=P, m=fd)
    dV3 = dV[:].rearrange("(n p m) -> n p m", p=P, m=fd)
    dw3 = dw[:].rearrange("(n p m) -> n p m", p=P, m=fd)

    # Exp's bias must be a per-partition SBUF AP (walrus requirement for
    # non-Copy activations); memset one before the Tile region, like Bass's
    # own const-AP registration does.
    bexp_t = nc.alloc_sbuf_tensor("const-bexp", [P, 1], f32)
    nc.gpsimd.memset(bexp_t.ap(), b_exp)
    nc.all_engine_barrier()
    b_exp_ap = bexp_t.ap()

    with TileContext(nc) as tc:
        with tc.tile_pool(name="pool", bufs=bufs) as pool:

            def body():
                for i in range(ntiles):
                    vt = pool.tile([P, fd], f32)
                    nc.sync.dma_start(out=vt[:, :], in_=V3[i, :, :])
                    wt = pool.tile([P, fd], f32)
                    nc.sync.dma_start(out=wt[:, :], in_=w3[i, :, :])

                    if mode == "memcpy":
                        nc.sync.dma_start(out=dV3[i, :, :], in_=vt[:, :])
                        nc.sync.dma_start(out=dw3[i, :, :], in_=wt[:, :])
                        continue

                    # E = (delta_T/tau) * exp((V-V_T)/delta_T)   [ScalarE]
                    et = pool.tile([P, fd], f32)
                    nc.scalar.activation(
                        et[:, :], vt[:, :], AF.Exp, bias=b_exp_ap, scale=s_exp
                    )
                    # at = alpha*V + gamma                        [DVE TS 2x]
                    at = pool.tile([P, fd], f32)
                    nc.vector.tensor_scalar(
                        at[:, :], vt[:, :], alpha, b_q, OP.mult, OP.add
                    )
                    # at += E                                     [DVE TT 1x]
                    nc.vector.tensor_add(out=at[:, :], in0=at[:, :], in1=et[:, :])
                    # et := beta*w  (reuse et slot)               [ScalarE]
                    nc.scalar.activation(
                        et[:, :], wt[:, :], AF.Copy, bias=0.0, scale=s_q
                    )
                    # at += beta*w → dVdt                         [DVE TT 1x]
                    nc.vector.tensor_add(out=at[:, :], in0=at[:, :], in1=et[:, :])
                    # vt := a2*V + c2w  (in-place; V fully consumed) [DVE TS 2x]
                    nc.vector.tensor_scalar(
                        vt[:, :], vt[:, :], a2, c_w2, OP.mult, OP.add
                    )
                    # wt := b2*w  (in-place; w fully consumed)    [ScalarE]
                    nc.scalar.activation(
                        wt[:, :], wt[:, :], AF.Copy, bias=0.0, scale=b_w2
                    )
                    # vt += b2*w → dwdt                           [DVE TT 1x]
                    nc.vector.tensor_add(out=vt[:, :], in0=vt[:, :], in1=wt[:, :])

                    nc.sync.dma_start(out=dV3[i, :, :], in_=at[:, :])
                    nc.sync.dma_start(out=dw3[i, :, :], in_=vt[:, :])

            if repeat == 1:
                body()
            else:
                with tc.For_i(0, repeat, 1):
                    body()

    if not nc.is_finalized():
        nc.finalize()  # Bacc.finalize runs compile() (reg alloc, wait splitting)
    _BUILT[key] = nc
    return nc


def _fold_constants(inputs):
    t = np.asarray(inputs["t"], dtype=np.float32)
    I_ext = np.asarray(inputs["I_ext"], dtype=np.float32)
    scal = lambda k: float(np.asarray(inputs[k]).reshape(-1)[0])
    V_rest, V_T, delta_T = scal("V_rest"), scal("V_T"), scal("delta_T")
    R, tau, tau_w, a = scal("R"), scal("tau"), scal("tau_w"), scal("a")

    # idx exactly as the reference: floor(t[0]/I_BIN) in f32
    idx = int(np.floor(np.divide(t[0], np.float32(I_BIN), dtype=np.float32)))
    I_t = float(I_ext[idx])

    s_exp = 1.0 / delta_T
    b_exp = -V_T / delta_T + math.log(delta_T / tau)
    alpha = -1.0 / tau
    beta = -R / tau
    gamma = (V_rest + R * I_t) / tau
    a2 = a / tau_w
    b2 = -1.0 / tau_w
    c2w = -a * V_rest / tau_w

    row = np.array([s_exp, b_exp, b2, c2w, beta, gamma, a2, alpha], dtype=np.float32)
    return tuple(float(x) for x in row)


def run(inputs, trace=False, **kwargs):
    """Compile+run on 8 cores; returns ((dVdt, dwdt), BassKernelResults)."""
    from concourse.bass_utils import run_bass_kernel_spmd

    V = np.ascontiguousarray(np.asarray(inputs["V"], dtype=np.float32))
    w = np.ascontiguousarray(np.asarray(inputs["w"], dtype=np.float32))
    consts = _fold_constants(inputs)

    nc = _build(consts)
    in_maps = [
        {
            "V": V[c * NSHARD : (c + 1) * NSHARD],
            "w": w[c * NSHARD : (c + 1) * NSHARD],
        }
        for c in range(NCORES)
    ]
    res = run_bass_kernel_spmd(nc, in_maps, list(range(NCORES)), trace=trace, **kwargs)
    dVdt = np.concatenate([res.results[c]["dVdt"] for c in range(NCORES)])
    dwdt = np.concatenate([res.results[c]["dwdt"] for c in range(NCORES)])
    return (dVdt, dwdt), res


def kernel(**inputs):
    out, _ = run(inputs, trace=False)
    return out


def make_exec_fn(consts, repeat=1, mode="full", fd=None, bufs=3):
    """Build a reusable jitted executor over pre-sharded device arrays.

    Returns (fn, mesh, arg_names): fn(*dev_arrays) -> outputs; arg order is
    V_full, w_full, dV_zeros, dw_zeros (each a full [N] array sharded on
    axis 0 across the 8-core mesh). For slope benchmarking only.
    """
    import jax
    from jax.experimental.shard_map import shard_map
    from jax.sharding import Mesh, PartitionSpec

    from concourse import bass2jax, mybir

    bass2jax.install_neuronx_cc_hook()
    nc = _build(consts, repeat, mode, fd=fd, bufs=bufs)

    partition_name = nc.partition_id_tensor.name if nc.partition_id_tensor else None
    in_names, out_names, out_avals = [], [], []
    for alloc in nc.m.functions[0].allocations:
        if not isinstance(alloc, mybir.MemoryLocationSet):
            continue
        name = alloc.memorylocations[0].name
        if alloc.kind == "ExternalInput":
            if name != partition_name:
                in_names.append(name)
        elif alloc.kind == "ExternalOutput":
            out_names.append(name)
            out_avals.append(
                jax.core.ShapedArray(
                    tuple(alloc.tensor_shape), mybir.dt.np(alloc.dtype)
                )
            )
    all_in = list(in_names) + list(out_names)
    if partition_name is not None:
        all_in.append(partition_name)

    def _body(*args):
        operands = list(args)
        if partition_name is not None:
            operands.append(bass2jax.partition_id_tensor())
        outs = bass2jax._bass_exec_p.bind(
            *operands,
            out_avals=tuple(out_avals),
            in_names=tuple(all_in),
            out_names=tuple(out_names),
            lowering_input_output_aliases=(),
            sim_require_finite=True,
            sim_require_nnan=True,
            nc=nc,
        )
        return tuple(outs)

    devices = jax.devices()[: NCORES]
    mesh = Mesh(np.asarray(devices), ("core",))
    nargs = len(in_names) + len(out_names)
    fn = jax.jit(
        shard_map(
            _body,
            mesh=mesh,
            in_specs=(PartitionSpec("core"),) * nargs,
            out_specs=(PartitionSpec("core"),) * len(out_names),
            check_rep=False,
        ),
        keep_unused=True,
    )
    return fn, mesh, in_names + out_names


# revision 23
# speedup vs baseline: 1.4950x; 1.3516x over previous
"""AdEx neuron RHS on 8 Trainium2 NeuronCores (Bass/Tile, SPMD).

dVdt = (-(V - V_rest) + delta_T*exp((V - V_T)/delta_T) - R*w + R*I(t)) / tau
dwdt = (a*(V - V_rest) - w) / tau_w

All [1]-shaped params plus the I_ext(t) table lookup are folded on the host
into 8 scalar constants, so the device kernel is pure elementwise:

    E  = exp(s_exp*V + b_exp)          # == (delta_T/tau)*exp((V-V_T)/delta_T)
    dV = alpha*V + (beta*w + gamma) + E
    dw = a2*V + (b2*w + c2w)

Sharding: V/w (and both outputs) split evenly across 8 cores on axis 0;
the constants are replicated.
"""

import math

import numpy as np

N = 33554432
NCORES = 8
NSHARD = N // NCORES  # 4194304
P = 128
FD = 2048  # default free-dim elements per tile
I_BIN = 0.001

_BUILT = {}


def _build(consts, repeat=1, mode="full", fd=None, bufs=3):
    """consts: tuple of 8 f32 floats (s_exp, b_exp, b2, c2w, beta, gamma, a2, alpha).

    repeat>1 wraps the whole shard pass in a dynamic For_i loop (for slope
    benchmarking: per-pass time = d(wall)/d(repeat), immune to dispatch
    overhead). mode="memcpy" skips compute (DMA roundtrip probe)."""
    fd = FD if fd is None else fd
    key = (consts, repeat, mode, fd, bufs)
    if key in _BUILT:
        return _BUILT[key]
    ntiles = NSHARD // (P * fd)

    import concourse.bacc as bacc
    import concourse.mybir as mybir
    from concourse.tile import TileContext

    f32 = mybir.dt.float32
    AF = mybir.ActivationFunctionType
    OP = mybir.AluOpType
    s_exp, b_exp, b_w2, c_w2, s_q, b_q, a2, alpha = consts

    nc = bacc.Bacc(None)
    V = nc.declare_dram_parameter("V", [NSHARD], f32, isOutput=False)
    w = nc.declare_dram_parameter("w", [NSHARD], f32, isOutput=False)
    dV = nc.declare_dram_parameter("dVdt", [NSHARD], f32, isOutput=True)
    dw = nc.declare_dram_parameter("dwdt", [NSHARD], f32, isOutput=True)

    V3 = V[:].rearrange("(n p m) -> n p m", p=P, m=fd)
    w3 = w[:].rearrange("(n p m) -> n p m", p=P, m=fd)
    dV3 = dV[:].rearrange("(n p m) -> n p m", p=P, m=fd)
    dw3 = dw[:].rearrange("(n p m) -> n p m", p=P, m=fd)

    # Exp's bias must be a per-partition SBUF AP (walrus requirement for
    # non-Copy activations); memset one before the Tile region, like Bass's
    # own const-AP registration does.
    bexp_t = nc.alloc_sbuf_tensor("const-bexp", [P, 1], f32)
    nc.gpsimd.memset(bexp_t.ap(), b_exp)
    nc.all_engine_barrier()
    b_exp_ap = bexp_t.ap()

    with TileContext(nc) as tc:
        with tc.tile_pool(name="pool", bufs=bufs) as pool:

            def body():
                for i in range(ntiles):
                    vt = pool.tile([P, fd], f32)
                    wt = pool.tile([P, fd], f32)
                    if mode != "compute":
                        nc.sync.dma_start(out=vt[:, :], in_=V3[i, :, :])
                        nc.sync.dma_start(out=wt[:, :], in_=w3[i, :, :])

                    if mode == "memcpy":
                        nc.sync.dma_start(out=dV3[i, :, :], in_=vt[:, :])
                        nc.sync.dma_start(out=dw3[i, :, :], in_=wt[:, :])
                        continue

                    # E = (delta_T/tau) * exp((V-V_T)/delta_T)   [ScalarE]
                    et = pool.tile([P, fd], f32)
                    nc.scalar.activation(
                        et[:, :], vt[:, :], AF.Exp, bias=b_exp_ap, scale=s_exp
                    )
                    # at = alpha*V + gamma                        [DVE TS 2x]
                    at = pool.tile([P, fd], f32)
                    nc.vector.tensor_scalar(
                        at[:, :], vt[:, :], alpha, b_q, OP.mult, OP.add
                    )
                    # at += E                                     [DVE TT 1x]
                    nc.vector.tensor_add(out=at[:, :], in0=at[:, :], in1=et[:, :])
                    # et := beta*w  (reuse et slot)               [ScalarE]
                    nc.scalar.activation(
                        et[:, :], wt[:, :], AF.Copy, bias=0.0, scale=s_q
                    )
                    # at += beta*w → dVdt                         [DVE TT 1x]
                    nc.vector.tensor_add(out=at[:, :], in0=at[:, :], in1=et[:, :])
                    # vt := a2*V + c2w  (in-place; V fully consumed) [DVE TS 2x]
                    nc.vector.tensor_scalar(
                        vt[:, :], vt[:, :], a2, c_w2, OP.mult, OP.add
                    )
                    # wt := b2*w  (in-place; w fully consumed)    [ScalarE]
                    nc.scalar.activation(
                        wt[:, :], wt[:, :], AF.Copy, bias=0.0, scale=b_w2
                    )
                    # vt += b2*w → dwdt                           [DVE TT 1x]
                    nc.vector.tensor_add(out=vt[:, :], in0=vt[:, :], in1=wt[:, :])

                    if mode != "compute":
                        nc.sync.dma_start(out=dV3[i, :, :], in_=at[:, :])
                        nc.sync.dma_start(out=dw3[i, :, :], in_=vt[:, :])

            if repeat == 1:
                body()
            else:
                with tc.For_i(0, repeat, 1):
                    body()

    if not nc.is_finalized():
        nc.finalize()  # Bacc.finalize runs compile() (reg alloc, wait splitting)
    _BUILT[key] = nc
    return nc


def _fold_constants(inputs):
    t = np.asarray(inputs["t"], dtype=np.float32)
    I_ext = np.asarray(inputs["I_ext"], dtype=np.float32)
    scal = lambda k: float(np.asarray(inputs[k]).reshape(-1)[0])
    V_rest, V_T, delta_T = scal("V_rest"), scal("V_T"), scal("delta_T")
    R, tau, tau_w, a = scal("R"), scal("tau"), scal("tau_w"), scal("a")

    # idx exactly as the reference: floor(t[0]/I_BIN) in f32
    idx = int(np.floor(np.divide(t[0], np.float32(I_BIN), dtype=np.float32)))
    I_t = float(I_ext[idx])

    s_exp = 1.0 / delta_T
    b_exp = -V_T / delta_T + math.log(delta_T / tau)
    alpha = -1.0 / tau
    beta = -R / tau
    gamma = (V_rest + R * I_t) / tau
    a2 = a / tau_w
    b2 = -1.0 / tau_w
    c2w = -a * V_rest / tau_w

    row = np.array([s_exp, b_exp, b2, c2w, beta, gamma, a2, alpha], dtype=np.float32)
    return tuple(float(x) for x in row)


def run(inputs, trace=False, **kwargs):
    """Compile+run on 8 cores; returns ((dVdt, dwdt), BassKernelResults)."""
    from concourse.bass_utils import run_bass_kernel_spmd

    V = np.ascontiguousarray(np.asarray(inputs["V"], dtype=np.float32))
    w = np.ascontiguousarray(np.asarray(inputs["w"], dtype=np.float32))
    consts = _fold_constants(inputs)

    nc = _build(consts)
    in_maps = [
        {
            "V": V[c * NSHARD : (c + 1) * NSHARD],
            "w": w[c * NSHARD : (c + 1) * NSHARD],
        }
        for c in range(NCORES)
    ]
    res = run_bass_kernel_spmd(nc, in_maps, list(range(NCORES)), trace=trace, **kwargs)
    dVdt = np.concatenate([res.results[c]["dVdt"] for c in range(NCORES)])
    dwdt = np.concatenate([res.results[c]["dwdt"] for c in range(NCORES)])
    return (dVdt, dwdt), res


def kernel(**inputs):
    out, _ = run(inputs, trace=False)
    return out


def make_exec_fn(consts, repeat=1, mode="full", fd=None, bufs=3):
    """Build a reusable jitted executor over pre-sharded device arrays.

    Returns (fn, mesh, arg_names): fn(*dev_arrays) -> outputs; arg order is
    V_full, w_full, dV_zeros, dw_zeros (each a full [N] array sharded on
    axis 0 across the 8-core mesh). For slope benchmarking only.
    """
    import jax
    from jax.experimental.shard_map import shard_map
    from jax.sharding import Mesh, PartitionSpec

    from concourse import bass2jax, mybir

    bass2jax.install_neuronx_cc_hook()
    nc = _build(consts, repeat, mode, fd=fd, bufs=bufs)

    partition_name = nc.partition_id_tensor.name if nc.partition_id_tensor else None
    in_names, out_names, out_avals = [], [], []
    for alloc in nc.m.functions[0].allocations:
        if not isinstance(alloc, mybir.MemoryLocationSet):
            continue
        name = alloc.memorylocations[0].name
        if alloc.kind == "ExternalInput":
            if name != partition_name:
                in_names.append(name)
        elif alloc.kind == "ExternalOutput":
            out_names.append(name)
            out_avals.append(
                jax.core.ShapedArray(
                    tuple(alloc.tensor_shape), mybir.dt.np(alloc.dtype)
                )
            )
    all_in = list(in_names) + list(out_names)
    if partition_name is not None:
        all_in.append(partition_name)

    def _body(*args):
        operands = list(args)
        if partition_name is not None:
            operands.append(bass2jax.partition_id_tensor())
        outs = bass2jax._bass_exec_p.bind(
            *operands,
            out_avals=tuple(out_avals),
            in_names=tuple(all_in),
            out_names=tuple(out_names),
            lowering_input_output_aliases=(),
            sim_require_finite=True,
            sim_require_nnan=True,
            nc=nc,
        )
        return tuple(outs)

    devices = jax.devices()[: NCORES]
    mesh = Mesh(np.asarray(devices), ("core",))
    nargs = len(in_names) + len(out_names)
    fn = jax.jit(
        shard_map(
            _body,
            mesh=mesh,
            in_specs=(PartitionSpec("core"),) * nargs,
            out_specs=(PartitionSpec("core"),) * len(out_names),
            check_rep=False,
        ),
        keep_unused=True,
    )
    return fn, mesh, in_names + out_names
